# revision 2
# baseline (speedup 1.0000x reference)
"""DotAttackHead kernel for Trainium2 (8 NeuronCores, data-parallel over batch).

prob = softmax(relu(ufeat @ W.T + b) @ efeat.T / sqrt(256) + mask_bias)
W = g * v / ||v||_F

Sharding: batch 64 -> 8 cores x 8 batches (data-parallel). Params replicated.

Host prep: weight-norm W, transpose+bf16-cast of ufeat/efeat (the PE needs
the contraction dim on partitions, and bf16 halves input DMA), and the mask
folded into efeat: masked columns (n >= num_enemy) are set to -1e30, so
masked logits land at <= -1e28 and exp underflows to exactly 0 — the same 0
the reference's -1e9 bias produces.

Device per batch b (software-pipelined across batches):
  mm1:  projT[e,u] = relu(wT.T @ ufT[b] + bias)   (PE bf16; bias+relu fused
        on DVE as tensor_scalar add/max reading PSUM, bf16 out)
  mm2:  psum[u,n]  = projT.T @ efT[b]             (PE bf16, fp32 PSUM)
  soft: et = Exp(psum/16) (ACT, bf16, written directly into the ganged
        [128,4,W] store tile), row-sum s on DVE tensor_reduce (f32),
        bf16 et DMA out per 4 u-tiles; all 64 row-sum tiles gang into one
        [128, bpc, 8] tile DMA'd once at the end. The softmax division
        happens on the HOST during the f32 upcast (prob = et / s), which
        removes the reciprocal+multiply (35us of DVE) and the ACT
        accumulator reads (14us) from the device's critical path.
No max-subtraction: logits are O(+-6) so exp is safe in fp32, and softmax is
shift-invariant, so this matches the reference.

Mask-width specialization: masked output columns are exactly 0, so the
program is compiled (per num_enemy multiset, NEFF-cached) with a static
per-slot column budget: batches sorted by effective width descending,
rank 8k+c -> (core c, slot k), slot width = slot max rounded up to 128.
Only columns [0, W_k) are computed/stored; the rest of each output row is
zeroed on host.

fp8 was evaluated for the matmuls (DoubleRow, 2x PE) and REJECTED: e4m3
quantization of proj+efeat pushes rel err to 4e-2 > the 2e-2 gate.
"""

from contextlib import ExitStack

import ml_dtypes
import numpy as np

import concourse.bass as bass
import concourse.mybir as mybir
import concourse.tile as tile
from concourse import bacc
from concourse.bass_utils import run_bass_kernel_spmd

N_CORES = 8
B = 64
U = 1024  # units
E = 256   # efeat dim
K = 512   # ufeat dim
N = 1024  # enemies
BPC = B // N_CORES  # batches per core

F32 = mybir.dt.float32
BF16 = mybir.dt.bfloat16
BF16_NP = ml_dtypes.bfloat16

def _build_bass(bpc: int = BPC, widths: tuple = ()) -> bass.Bass:
    if not widths:
        widths = (N,) * bpc
    assert len(widths) == bpc and all(w % 128 == 0 and 128 <= w <= N for w in widths)
    # Bacc (not raw Bass): its finalize() runs generate_event_semaphores,
    # which splits multi-wait instructions to satisfy TRN2's 1-wait limit.
    nc = bacc.Bacc(None, target_bir_lowering=False)

    ufT = nc.declare_dram_parameter("ufT", [bpc, K, U], BF16, isOutput=False)
    efT = nc.declare_dram_parameter("efT", [bpc, E, N], BF16, isOutput=False)
    wT = nc.declare_dram_parameter("wT", [K, E], BF16, isOutput=False)
    bias = nc.declare_dram_parameter("bias", [E], F32, isOutput=False)
    # bf16 unnormalized exp out + f32 row sums; host does prob = et / s.
    prob = nc.declare_dram_parameter("prob", [bpc, U, N], BF16, isOutput=True)
    sums = nc.declare_dram_parameter("sums", [bpc, 8, 128], F32, isOutput=True)

    with tile.TileContext(nc) as tc, ExitStack() as ctx:
        singles = ctx.enter_context(tc.tile_pool(name="singles", bufs=1))
        pin = ctx.enter_context(tc.tile_pool(name="pin", bufs=5))
        pproj = ctx.enter_context(tc.tile_pool(name="pproj", bufs=3))
        pprob = ctx.enter_context(tc.tile_pool(name="pprob", bufs=4))
        pps1 = ctx.enter_context(tc.tile_pool(name="pps1", bufs=2, space="PSUM"))
        pps2 = ctx.enter_context(tc.tile_pool(name="pps2", bufs=3, space="PSUM"))

        # ---- resident constants ----
        # wT as 4 k-tiles: wt_sb[p, kt, e] = wT[kt*128+p, e]
        wt_sb = singles.tile([128, 4, E], BF16)
        nc.sync.dma_start(out=wt_sb, in_=wT[:, :].rearrange("(kt p) e -> p kt e", p=128))
        # bias as 2 e-tiles on partitions: b_sb[p, et] = bias[et*128+p]
        b_sb = singles.tile([128, 2], F32)
        nc.sync.dma_start(out=b_sb, in_=bias[:].rearrange("(et p) -> p et", p=128))
        # all 64 row-sum scalars gang into one tile, one DMA at the end
        s_all = singles.tile([128, bpc, 8], F32)

        def emit_loads(bi):
            uft = pin.tile([128, 4, U], BF16, tag="uft")
            if bi == 0:
                # two u-half loads: mm1's first (uc=0) groups start after
                # 512KB lands instead of the full 1MB (ramp only matters
                # for batch 0; later batches are prefetched)
                for uc in range(2):
                    usl = slice(uc * 512, (uc + 1) * 512)
                    nc.sync.dma_start(
                        out=uft[:, :, usl],
                        in_=ufT[bi, :, usl].rearrange("(kt p) u -> p kt u", p=128),
                    )
            else:
                nc.sync.dma_start(
                    out=uft, in_=ufT[bi, :, :].rearrange("(kt p) u -> p kt u", p=128)
                )
            W = widths[bi]
            eft = pin.tile([128, 2, W], BF16, tag="eft", name=f"eft{bi}")
            nc.sync.dma_start(
                out=eft, in_=efT[bi, :, :W].rearrange("(et p) n -> p et n", p=128)
            )
            return uft, eft

        def emit_mm1_group(uft, projT, gi):
            # group gi -> (ej, uc), uc-major: both e-halves of u-chunk 0 come
            # first, so mm2 tiles u0..u3 unblock after 2 groups instead of 4
            ej, uc = gi % 2, gi // 2
            esl = slice(ej * 128, (ej + 1) * 128)
            usl = slice(uc * 512, (uc + 1) * 512)
            ps1 = pps1.tile([128, 512], F32, tag="ps1")
            for kj in range(4):
                nc.tensor.matmul(
                    ps1,
                    lhsT=wt_sb[:, kj, esl],
                    rhs=uft[:, kj, usl],
                    start=(kj == 0),
                    stop=(kj == 3),
                )
            # relu(x + b) = max(x + b, 0) fused on DVE; casts to bf16
            nc.vector.tensor_scalar(
                out=projT[:, ej, usl],
                in0=ps1,
                scalar1=b_sb[:, ej : ej + 1],
                scalar2=0.0,
                op0=mybir.AluOpType.add,
                op1=mybir.AluOpType.max,
            )

        pair_state = {}

        def emit_softmax_tile(bi, projT, eft, ui):
            # only the first widths[bi] columns are live (the rest of the
            # output row is zeroed on host)
            W = widths[bi]
            nslices = [slice(0, min(512, W))] + ([slice(512, W)] if W > 512 else [])
            uslice = slice(ui * 128, (ui + 1) * 128)
            ps2 = pps2.tile([128, W], F32, tag="ps2", name=f"ps2_{bi}_{ui}")
            # e-major: consecutive matmuls share the same lhsT (weight reuse)
            for ej in range(2):
                for nsl in nslices:
                    nc.tensor.matmul(
                        ps2[:, nsl],
                        lhsT=projT[:, ej, uslice],
                        rhs=eft[:, ej, nsl],
                        start=(ej == 0),
                        stop=(ej == 1),
                    )
            # gang 4 adjacent u-tiles into one [128, 4, W] store tile:
            # quarters the Sync DIRECT2D issue count (per-DMA cost is
            # size-independent) and batches the output into 1-2MB transfers
            if ui % 4 == 0:
                pair_state["tile"] = pprob.tile(
                    [128, 4, W], BF16, tag="prob", name=f"prob{bi}_{ui}"
                )
            prob_t = pair_state["tile"]
            # unnormalized exp straight into the store tile (bf16)
            nc.scalar.activation(
                out=prob_t[:, ui % 4, :],
                in_=ps2,
                func=mybir.ActivationFunctionType.Exp,
                scale=1.0 / 16.0,
            )
            # row sum on DVE (f32 out); host divides during the upcast
            nc.vector.tensor_reduce(
                out=s_all[:, bi, ui : ui + 1],
                in_=prob_t[:, ui % 4, :],
                axis=mybir.AxisListType.X,
                op=mybir.AluOpType.add,
            )
            if ui % 4 == 3:
                base = (ui - 3) * 128
                nc.sync.dma_start(
                    out=prob[bi, base : base + 512, :W].rearrange(
                        "(j p) n -> p j n", p=128
                    ),
                    in_=prob_t,
                )

        # Software-pipelined emission: mm1 groups for batch bi+1 are emitted
        # between softmax tiles of batch bi's second half, so the PE never
        # monopolizes a contiguous ~4us window on mm1 while ACT's 3-deep
        # PSUM backlog drains.
        tiles = {0: emit_loads(0)}
        projs = {0: pproj.tile([128, 2, U], BF16, tag="projT", name="projT0")}
        for gi in range(4):
            emit_mm1_group(tiles[0][0], projs[0], gi)
        for bi in range(bpc):
            uft, eft = tiles[bi]
            projT = projs[bi]
            if bi + 1 < bpc:
                tiles[bi + 1] = emit_loads(bi + 1)
            for ui in range(4):
                emit_softmax_tile(bi, projT, eft, ui)
            if bi + 1 < bpc:
                projs[bi + 1] = pproj.tile(
                    [128, 2, U], BF16, tag="projT", name=f"projT{bi + 1}"
                )
            # mm1 groups for bi+1 ride along u4..u7 so the PE never
            # monopolizes a contiguous ~4us window on mm1 while ACT's
            # 3-deep PSUM backlog drains
            for ui in range(4, 8):
                emit_softmax_tile(bi, projT, eft, ui)
                if bi + 1 < bpc:
                    emit_mm1_group(tiles[bi + 1][0], projs[bi + 1], ui - 4)
        # one tiny DMA for all 64 row sums
        nc.sync.dma_start(
            out=sums[:, :, :].rearrange("b ui p -> p b ui"), in_=s_all
        )

    # Runs Bacc.compile(): register allocation + event-semaphore splitting.
    nc.finalize()
    return nc


def _prep_inputs(ufeat, efeat, num_enemy, v, g, b):
    """Host-side prep: weight-norm, transpose + bf16 cast, mask bias."""
    ufeat = np.asarray(ufeat, dtype=np.float32)
    efeat = np.asarray(efeat, dtype=np.float32)
    num_enemy = np.asarray(num_enemy).astype(np.int64)
    v = np.asarray(v, dtype=np.float32)
    g = np.float32(np.asarray(g))
    b = np.asarray(b, dtype=np.float32)

    W = (g / np.float32(np.linalg.norm(v))) * v  # [E, K]
    wT = np.ascontiguousarray(W.T).astype(BF16_NP)  # [K, E]

    # [B, K, U] / [B, E, N] bf16 (cast first: halves the transpose traffic)
    ufT = ufeat.astype(BF16_NP).transpose(0, 2, 1)
    efT = np.ascontiguousarray(efeat.astype(BF16_NP).transpose(0, 2, 1))

    # Mask: poison masked efeat columns (n >= num_enemy) with -1e30. Since
    # proj >= 0 (relu) and a proj row is never identically 0 in practice,
    # masked logits land at <= -1e28 and exp underflows to exactly 0 — the
    # same 0 the reference's -1e9 bias produces. num_enemy==0 => all lanes
    # masked => the reference's uniform -1e9 shift cancels in softmax =>
    # leave those batches unpoisoned.
    ne = np.where(num_enemy > 0, num_enemy, N)
    col_masked = np.arange(N)[None, :] >= ne[:, None]  # [B, N]
    efT[np.broadcast_to(col_masked[:, None, :], efT.shape)] = BF16_NP(-1e30)

    return ufT, efT, wT, b


_nc_cache: dict[tuple, bass.Bass] = {}


def run(ufeat, efeat, num_enemy, v, g, b, trace=False):
    ufT, efT, wT, b = _prep_inputs(ufeat, efeat, num_enemy, v, g, b)

    # Masked columns (n >= num_enemy) of the output are exactly 0, so the
    # kernel only needs to compute/store columns [0, W) per batch. Sort
    # batches by effective width (descending), assign rank 8k+c to
    # (core c, slot k), and compile the program with a static per-slot
    # width = the slot's max rounded up to 128. Identical widths across
    # cores keeps it SPMD.
    ne = np.asarray(num_enemy).astype(np.int64)
    ne_eff = np.where(ne > 0, ne, N)
    order = np.argsort(-ne_eff, kind="stable")  # descending: widest slot
    # first (overlaps the ramp), narrowest last (short drain tail)
    slot_ne = ne_eff[order].reshape(BPC, N_CORES)
    widths = tuple(
        int(max(128, -(-int(m) // 128) * 128)) for m in slot_ne.max(axis=1)
    )

    key = (BPC, widths)
    if key not in _nc_cache:
        _nc_cache[key] = _build_bass(BPC, widths)
    nc = _nc_cache[key]

    in_maps = []
    perms = []
    for c in range(N_CORES):
        perm = order.reshape(BPC, N_CORES)[:, c]  # batch index for each slot
        perms.append(perm)
        in_maps.append({"ufT": ufT[perm], "efT": efT[perm], "wT": wT, "bias": b})

    res = run_bass_kernel_spmd(nc, in_maps, list(range(N_CORES)), trace=trace)
    out = np.empty((B, U, N), dtype=np.float32)
    for c in range(N_CORES):
        o = res.results[c]["prob"].astype(np.float32)  # [bpc, U, N]
        s = np.asarray(res.results[c]["sums"], dtype=np.float32)  # [bpc, 8, 128]
        o /= s.reshape(BPC, U, 1)  # row u = ui*128+p  <->  s[k, ui, p]
        # Columns [W, N) are masked => exactly 0. The device never writes
        # them; zero here so correctness never rests on buffer-init
        # behavior (and to wipe any NaN from the divide of garbage).
        for k, w in enumerate(widths):
            o[k, :, w:] = 0.0
        out[perms[c]] = o
    return out, res


def kernel(ufeat, efeat, num_enemy, v, g, b):
    out, _ = run(ufeat, efeat, num_enemy, v, g, b, trace=False)
    return out


# revision 4
# speedup vs baseline: 1.0819x; 1.0819x over previous
"""DotAttackHead kernel for Trainium2 (8 NeuronCores, data-parallel over batch).

prob = softmax(relu(ufeat @ W.T + b) @ efeat.T / sqrt(256) + mask_bias)
W = g * v / ||v||_F

Sharding: batch 64 -> 8 cores x 8 batches (data-parallel). Params replicated.

Host prep: weight-norm W, bf16 cast, mask folded into efeat (masked columns
poisoned with -1e30 so masked logits underflow exp to exactly 0), and BOTH
inputs staged in the exact SBUF tile layout ([p, kt, u] / [p, et, n]) so
every load is a pure linear 128-partition DMA with 4-8KB contiguous
per-partition chunks (the naive [K, U]-transposing load produces 1KB
descriptors and runs at ~1/4 line rate).

Device per batch b (software-pipelined across batches):
  mm1:  projT[e,u] = relu(wT.T @ ufT[b] + bias)   (PE bf16; bias+relu fused
        on DVE as tensor_scalar add/max reading PSUM, bf16 out)
  mm2:  psum[u,n]  = projT.T @ efT[b]             (PE bf16, fp32 PSUM)
  soft: et = Exp(psum/16) with accum_out row-sum (ACT, bf16, written
        directly into the ganged [128,4,W] store tile). The softmax
        division happens on the HOST during the f32 upcast (prob = et / s),
        which removes the reciprocal+multiply (35us of DVE) from the device.
Row sums gang into one [128, bpc, 8] tile, DMA'd once at the end.

Output is stored in a COMPACT partition-major layout: one flat bf16 tensor,
per (slot, 4-u-tile gang) a [128, 4, W] block written as one linear DMA
(4W*2B contiguous per partition -> max-size descriptors). The host
un-permutes to [U, N], divides by the row sums and zero-fills columns
[W, N). Masked-width specialization: batches sorted by effective width
descending, rank 8k+c -> (core c, slot k), slot width = slot max rounded
up to 128, compiled per-widths (NEFF-cached).

No max-subtraction: logits are O(+-6) so exp is safe in fp32, and softmax is
shift-invariant, so this matches the reference.

fp8 was evaluated for the matmuls (DoubleRow, 2x PE) and REJECTED: e4m3
quantization of proj+efeat pushes rel err to 4e-2 > the 2e-2 gate.
"""

from contextlib import ExitStack

import ml_dtypes
import numpy as np

import concourse.bass as bass
import concourse.mybir as mybir
import concourse.tile as tile
from concourse import bacc
from concourse.bass_utils import run_bass_kernel_spmd

N_CORES = 8
B = 64
U = 1024  # units
E = 256   # efeat dim
K = 512   # ufeat dim
N = 1024  # enemies
BPC = B // N_CORES  # batches per core

F32 = mybir.dt.float32
BF16 = mybir.dt.bfloat16
BF16_NP = ml_dtypes.bfloat16

def _build_bass(bpc: int = BPC, widths: tuple = ()) -> bass.Bass:
    if not widths:
        widths = (N,) * bpc
    assert len(widths) == bpc and all(w % 128 == 0 and 128 <= w <= N for w in widths)
    # Bacc (not raw Bass): its finalize() runs generate_event_semaphores,
    # which splits multi-wait instructions to satisfy TRN2's 1-wait limit.
    nc = bacc.Bacc(None, target_bir_lowering=False)

    # inputs staged host-side in SBUF layout: linear 128-partition loads
    ufT = nc.declare_dram_parameter("ufT", [bpc, 128, 4 * U], BF16, isOutput=False)
    efT_sizes = [2 * w for w in widths]
    ef_off = np.cumsum([0] + efT_sizes)
    efT = nc.declare_dram_parameter(
        "efT", [128, int(ef_off[-1])], BF16, isOutput=False
    )
    wT = nc.declare_dram_parameter("wT", [K, E], BF16, isOutput=False)
    bias = nc.declare_dram_parameter("bias", [E], F32, isOutput=False)
    # compact partition-major output: per (slot, gang) a [128, 4, W] block,
    # linear per partition. Host un-permutes + divides by row sums.
    pr_sizes = [4 * w for w in widths]  # per-gang per-partition elems
    pr_off = np.cumsum([0] + [2 * s for s in pr_sizes])  # 2 gangs per slot
    prob = nc.declare_dram_parameter(
        "probc", [128, int(pr_off[-1])], BF16, isOutput=True
    )
    sums = nc.declare_dram_parameter("sums", [bpc, 8, 128], F32, isOutput=True)

    with tile.TileContext(nc) as tc, ExitStack() as ctx:
        singles = ctx.enter_context(tc.tile_pool(name="singles", bufs=1))
        pin = ctx.enter_context(tc.tile_pool(name="pin", bufs=5))
        pproj = ctx.enter_context(tc.tile_pool(name="pproj", bufs=3))
        pprob = ctx.enter_context(tc.tile_pool(name="pprob", bufs=4))
        pps1 = ctx.enter_context(tc.tile_pool(name="pps1", bufs=2, space="PSUM"))
        pps2 = ctx.enter_context(tc.tile_pool(name="pps2", bufs=3, space="PSUM"))

        # ---- resident constants ----
        # wT as 4 k-tiles: wt_sb[p, kt, e] = wT[kt*128+p, e]
        wt_sb = singles.tile([128, 4, E], BF16)
        nc.sync.dma_start(out=wt_sb, in_=wT[:, :].rearrange("(kt p) e -> p kt e", p=128))
        # bias as 2 e-tiles on partitions: b_sb[p, et] = bias[et*128+p]
        b_sb = singles.tile([128, 2], F32)
        nc.sync.dma_start(out=b_sb, in_=bias[:].rearrange("(et p) -> p et", p=128))
        # all 64 row-sum scalars gang into one tile, one DMA at the end
        s_all = singles.tile([128, bpc, 8], F32)

        def emit_loads(bi):
            uft = pin.tile([128, 4, U], BF16, tag="uft")
            if bi == 0:
                # two half loads: mm1's first groups start after 512KB lands
                for uc in range(2):
                    nc.sync.dma_start(
                        out=uft[:, :, uc * 512 : (uc + 1) * 512],
                        in_=ufT[bi, :, :].rearrange(
                            "p (kt u) -> p kt u", kt=4
                        )[:, :, uc * 512 : (uc + 1) * 512],
                    )
            else:
                nc.sync.dma_start(
                    out=uft,
                    in_=ufT[bi, :, :].rearrange("p (kt u) -> p kt u", kt=4),
                )
            W = widths[bi]
            eft = pin.tile([128, 2, W], BF16, tag="eft", name=f"eft{bi}")
            nc.sync.dma_start(
                out=eft,
                in_=efT[:, int(ef_off[bi]) : int(ef_off[bi + 1])].rearrange(
                    "p (et n) -> p et n", et=2
                ),
            )
            return uft, eft

        def emit_mm1_group(uft, projT, gi):
            # group gi -> (ej, uc), uc-major: both e-halves of u-chunk 0 come
            # first, so mm2 tiles u0..u3 unblock after 2 groups instead of 4
            ej, uc = gi % 2, gi // 2
            esl = slice(ej * 128, (ej + 1) * 128)
            usl = slice(uc * 512, (uc + 1) * 512)
            ps1 = pps1.tile([128, 512], F32, tag="ps1")
            for kj in range(4):
                nc.tensor.matmul(
                    ps1,
                    lhsT=wt_sb[:, kj, esl],
                    rhs=uft[:, kj, usl],
                    start=(kj == 0),
                    stop=(kj == 3),
                )
            # relu(x + b) = max(x + b, 0) fused on DVE; casts to bf16
            nc.vector.tensor_scalar(
                out=projT[:, ej, usl],
                in0=ps1,
                scalar1=b_sb[:, ej : ej + 1],
                scalar2=0.0,
                op0=mybir.AluOpType.add,
                op1=mybir.AluOpType.max,
            )

        pair_state = {}

        def emit_softmax_tile(bi, projT, eft, ui):
            # only the first widths[bi] columns are live
            W = widths[bi]
            nslices = [slice(0, min(512, W))] + ([slice(512, W)] if W > 512 else [])
            uslice = slice(ui * 128, (ui + 1) * 128)
            ps2 = pps2.tile([128, W], F32, tag="ps2", name=f"ps2_{bi}_{ui}")
            # e-major: consecutive matmuls share the same lhsT (weight reuse)
            for ej in range(2):
                for nsl in nslices:
                    nc.tensor.matmul(
                        ps2[:, nsl],
                        lhsT=projT[:, ej, uslice],
                        rhs=eft[:, ej, nsl],
                        start=(ej == 0),
                        stop=(ej == 1),
                    )
            # gang 4 adjacent u-tiles into one [128, 4, W] store tile
            if ui % 4 == 0:
                pair_state["tile"] = pprob.tile(
                    [128, 4, W], BF16, tag="prob", name=f"prob{bi}_{ui}"
                )
            prob_t = pair_state["tile"]
            # unnormalized exp straight into the store tile (bf16) + row sum
            s = s_all[:, bi, ui : ui + 1]
            nc.scalar.activation(
                out=prob_t[:, ui % 4, :],
                in_=ps2,
                func=mybir.ActivationFunctionType.Exp,
                scale=1.0 / 16.0,
                accum_out=s,
            )
            if ui % 4 == 3:
                g = ui // 4
                off = int(pr_off[bi]) + g * 4 * W
                nc.sync.dma_start(
                    out=prob[:, off : off + 4 * W].rearrange(
                        "p (j n) -> p j n", j=4
                    ),
                    in_=prob_t,
                )

        # Software-pipelined emission: mm1 groups for batch bi+1 are emitted
        # between softmax tiles of batch bi's second half, so the PE never
        # monopolizes a contiguous ~4us window on mm1 while ACT's 3-deep
        # PSUM backlog drains.
        tiles = {0: emit_loads(0)}
        projs = {0: pproj.tile([128, 2, U], BF16, tag="projT", name="projT0")}
        for gi in range(4):
            emit_mm1_group(tiles[0][0], projs[0], gi)
        for bi in range(bpc):
            uft, eft = tiles[bi]
            projT = projs[bi]
            if bi + 1 < bpc:
                tiles[bi + 1] = emit_loads(bi + 1)
            for ui in range(4):
                emit_softmax_tile(bi, projT, eft, ui)
            if bi + 1 < bpc:
                projs[bi + 1] = pproj.tile(
                    [128, 2, U], BF16, tag="projT", name=f"projT{bi + 1}"
                )
            for ui in range(4, 8):
                emit_softmax_tile(bi, projT, eft, ui)
                if bi + 1 < bpc:
                    emit_mm1_group(tiles[bi + 1][0], projs[bi + 1], ui - 4)
        # one tiny DMA for all 64 row sums
        nc.sync.dma_start(
            out=sums[:, :, :].rearrange("b ui p -> p b ui"), in_=s_all
        )

    # Runs Bacc.compile(): register allocation + event-semaphore splitting.
    nc.finalize()
    return nc


def _widths_for(num_enemy):
    ne = np.asarray(num_enemy).astype(np.int64)
    ne_eff = np.where(ne > 0, ne, N)
    order = np.argsort(-ne_eff, kind="stable")
    slot_ne = ne_eff[order].reshape(BPC, N_CORES)
    widths = tuple(
        int(max(128, -(-int(m) // 128) * 128)) for m in slot_ne.max(axis=1)
    )
    return order, widths


def _prep_inputs(ufeat, efeat, num_enemy, v, g, b, widths):
    """Host prep: weight-norm, bf16 cast, mask poison, SBUF-layout staging."""
    ufeat = np.asarray(ufeat, dtype=np.float32)
    efeat = np.asarray(efeat, dtype=np.float32)
    num_enemy = np.asarray(num_enemy).astype(np.int64)
    v = np.asarray(v, dtype=np.float32)
    g = np.float32(np.asarray(g))
    b = np.asarray(b, dtype=np.float32)

    W = (g / np.float32(np.linalg.norm(v))) * v  # [E, K]
    wT = np.ascontiguousarray(W.T).astype(BF16_NP)  # [K, E]

    # SBUF layout: ufT[b, p, kt*U + u] = ufeat[b, u, kt*128+p]
    # [B, U, K] -> bf16 -> [B, K, U] -> [B, 4, 128, U] -> [B, 128, 4, U]
    ufT = (
        ufeat.astype(BF16_NP)
        .transpose(0, 2, 1)
        .reshape(B, 4, 128, U)
        .transpose(0, 2, 1, 3)
        .reshape(B, 128, 4 * U)
    )

    # efT in SBUF layout [B, 128, 2, N]: efT[b, p, et, n] = efeat[b, n, et*128+p]
    efT = efeat.astype(BF16_NP).transpose(0, 2, 1)  # [B, E, N]
    # Mask: poison masked efeat columns (n >= num_enemy) with -1e30. Since
    # proj >= 0 (relu) and a proj row is never identically 0 in practice,
    # masked logits land at <= -1e28 and exp underflows to exactly 0 — the
    # same 0 the reference's -1e9 bias produces. num_enemy==0 => all lanes
    # masked => uniform shift cancels in softmax => leave unpoisoned.
    ne = np.where(num_enemy > 0, num_enemy, N)
    col_masked = np.arange(N)[None, :] >= ne[:, None]  # [B, N]
    efT[np.broadcast_to(col_masked[:, None, :], efT.shape)] = BF16_NP(-1e30)
    efT = efT.reshape(B, 2, 128, N).transpose(0, 2, 1, 3)  # [B, 128, 2, N]

    return ufT, efT, wT, b


def _pack_ef(efT, perm, widths):
    """Per-core packed efeat: [128, sum(2*W)] per the compiled offsets."""
    cols = []
    for k, bi in enumerate(perm):
        w = widths[k]
        cols.append(efT[bi, :, :, :w].reshape(128, 2 * w))
    return np.ascontiguousarray(np.concatenate(cols, axis=1))


def _unpack_out(probc, s, widths):
    """[128, sum(8*W)] bf16 + [bpc, 8, 128] f32 sums -> [bpc, U, N] f32."""
    bpc = len(widths)
    out = np.zeros((bpc, U, N), dtype=np.float32)
    sinv = 1.0 / s.reshape(bpc, U)  # row u = ui*128+p
    off = 0
    for k, w in enumerate(widths):
        for gi in range(2):
            blk = probc[:, off : off + 4 * w].reshape(128, 4, w)
            # u = gi*512 + j*128 + p
            rows = blk.transpose(1, 0, 2).reshape(512, w).astype(np.float32)
            usl = slice(gi * 512, gi * 512 + 512)
            out[k, usl, :w] = rows * sinv[k, usl, None]
            off += 4 * w
    return out


_nc_cache: dict[tuple, bass.Bass] = {}


def run(ufeat, efeat, num_enemy, v, g, b, trace=False):
    order, widths = _widths_for(num_enemy)
    ufT, efT, wT, b = _prep_inputs(ufeat, efeat, num_enemy, v, g, b, widths)

    key = (BPC, widths)
    if key not in _nc_cache:
        _nc_cache[key] = _build_bass(BPC, widths)
    nc = _nc_cache[key]

    in_maps = []
    perms = []
    for c in range(N_CORES):
        perm = order.reshape(BPC, N_CORES)[:, c]  # batch index for each slot
        perms.append(perm)
        in_maps.append(
            {
                "ufT": np.ascontiguousarray(ufT[perm]),
                "efT": _pack_ef(efT, perm, widths),
                "wT": wT,
                "bias": b,
            }
        )

    res = run_bass_kernel_spmd(nc, in_maps, list(range(N_CORES)), trace=trace)
    out = np.empty((B, U, N), dtype=np.float32)
    for c in range(N_CORES):
        probc = np.asarray(res.results[c]["probc"])
        s = np.asarray(res.results[c]["sums"], dtype=np.float32)
        out[perms[c]] = _unpack_out(probc, s, widths)
    return out, res


def kernel(ufeat, efeat, num_enemy, v, g, b):
    out, _ = run(ufeat, efeat, num_enemy, v, g, b, trace=False)
    return out
